# revision 74
# baseline (speedup 1.0000x reference)
"""Trainium2 Bass kernel for multi-head causal attention with RoPE.

Problem: x[4,2048,1024] -> MHA(16 heads, head_dim 64, RoPE, causal) -> [4,2048,1024]

Sharding: 8 cores = 4 batches x 2 head-groups (8 heads each, Megatron-style).
Each core computes a partial [T, C] projection output for its batch; the host
sums the two head-group partials per batch and adds b_proj.

Per-core dataflow (all matmul moving operands are bf16 -> 1 cycle/row at any
width; stationary reloads are free in the cost model):
  - x is fed as bf16 and transposed HBM->SBUF by the DMA xbar (no PE/DVE cost)
  - Q^T/K^T in [d, t] layout (head-pair tiles on 128 partitions), RoPE via a
    host-precomputed perm matrix on PE + cos/sin muls on DVE, add on Pool;
    1/sqrt(64) folded into W_q host-side; result stored bf16
  - V in natural [t, d] layout bf16, head-major with a 65th column == 1.0
  - scores S^T = K Q^T per (head pair, 512-q chunk, 128-k block), causal
    skipping at 128-col granularity (ragged diagonal), batched exp on ACT,
    diagonal blocks masked post-exp on DVE (bf16 4x mode)
  - P@V in y[q, d] orientation: stationary = P^T block (free reload), moving =
    V65 (ap=65, denominator rides along in column 64); normalize with
    reciprocal + per-partition scalar mul; transpose y back with bf16
    identity (1 cyc/row); Pool copies y^T to SBUF
  - output projection y^T @ W_proj per 128-t chunk in two 512-col halves
  - phases A (proj), B (attention), C (out proj) are interleaved at work-unit
    granularity so B's ACT-bound exp stretches are covered by A/C PE work
"""

import math
import sys

import numpy as np

if "/opt/trn_rl_repo" not in sys.path:
    sys.path.insert(0, "/opt/trn_rl_repo")

import concourse.bass as bass
import concourse.tile as tile
from concourse import bacc
from concourse import mybir
from concourse.bass_utils import run_bass_kernel_spmd
from concourse.masks import make_identity

B, T, C = 4, 2048, 1024
NH, D = 16, 64
HL = 8              # local heads per core
DL = HL * D         # 512
NCORES = 8
P = 128
TCH = 512           # t-chunk width in phase A / q-chunk width in phase B
NTC = T // TCH
ROPE_BASE = 10000.0

F32 = mybir.dt.float32
F32R = mybir.dt.float32r
BF16 = mybir.dt.bfloat16
Exp = mybir.ActivationFunctionType.Exp


def _emit(tc, xb, wqk, wv, wp, cos2, sin2, bias, dmask, perm, out, dbg=None):
    nc = tc.nc
    with tc.tile_pool(name="pers", bufs=1) as pers, \
         tc.tile_pool(name="tmp", bufs=2) as ptmp, \
         tc.tile_pool(name="ptp", bufs=6) as ppt, \
         tc.tile_pool(name="ysp", bufs=4) as pys, \
         tc.tile_pool(name="rcp", bufs=2) as prcp, \
         tc.tile_pool(name="ost", bufs=2) as post, \
         tc.tile_pool(name="psa", bufs=2, space="PSUM") as psA, \
         tc.tile_pool(name="pss", bufs=2, space="PSUM") as psS, \
         tc.tile_pool(name="psy", bufs=2, space="PSUM") as psY:
        xT = pers.tile([P, 8, T], BF16)        # x^T: [c%128... c = cc*128+p]
        qkT = pers.tile([P, 8, T], BF16)       # j 0-3: Q pairs, 4-7: K pairs
        vsb = pers.tile([P, 16, HL * 65], BF16)  # [t%128, t//128, h*65 + e]
        yT = pers.tile([P, 4, T], BF16)        # [d2, g, t]
        wqk_sb = pers.tile([P, 8, 2 * DL], BF16)
        wv_sb = pers.tile([P, 8, DL], BF16)
        wp_sb = pers.tile([P, 4, C], BF16)
        cos_sb = pers.tile([P, T], F32)
        sin_sb = pers.tile([P, T], F32)
        bias_sb = pers.tile([P, 8 + DL], F32)
        dmask_sb = pers.tile([P, 1, P], BF16)  # [k, 1, q]: 1.0 iff k <= q
        perm_sb = pers.tile([P, P], F32R)
        identb = pers.tile([P, P], BF16)
        z128 = pers.tile([1, P], BF16)
        z260 = pers.tile([1, 260], BF16)

        make_identity(nc, identb)
        nc.vector.memset(z128[:], 0.0)
        nc.vector.memset(z260[:], 0.0)
        # DMA priority order: first x chunk + first weights before the rest so
        # PE can start within ~8us (the sim serializes all DMA on one resource)
        nc.sync.dma_start_transpose(xT[:, :, 0:TCH], xb[0:TCH, :])
        nc.sync.dma_start(
            wqk_sb[:, :, 0:DL], wqk[:, 0:DL].rearrange("(o p) n -> p o n", p=P))
        nc.sync.dma_start(cos_sb[:], cos2)
        nc.sync.dma_start(sin_sb[:], sin2)
        nc.sync.dma_start(
            wqk_sb[:, :, DL:], wqk[:, DL:].rearrange("(o p) n -> p o n", p=P))
        nc.sync.dma_start(wv_sb[:], wv.rearrange("(o p) n -> p o n", p=P))
        nc.sync.dma_start(bias_sb[:], bias)
        nc.sync.dma_start_transpose(xT[:, :, TCH:2 * TCH], xb[TCH:2 * TCH, :])
        nc.sync.dma_start(dmask_sb[:], dmask.rearrange("p (o w) -> p o w", o=1))
        nc.gpsimd.dma_start(perm_sb[:], perm)
        nc.sync.dma_start_transpose(
            xT[:, :, 2 * TCH:3 * TCH], xb[2 * TCH:3 * TCH, :])
        nc.sync.dma_start_transpose(
            xT[:, :, 3 * TCH:4 * TCH], xb[3 * TCH:4 * TCH, :])
        nc.sync.dma_start(wp_sb[:], wp.rearrange("(o p) n -> p o n", p=P))
        vg = vsb.rearrange("p a (h e) -> p a h e", e=65)
        nc.vector.memset(vg[:, :, :, 64:65], 1.0)

        # ---- phase A unit: one QK projection column-pair j of one t-chunk
        def gen_qk_j(tcn, j):
            ts0 = tcn * TCH
            # the whole psq accumulation stays in one step: a same-slot
            # allocation from another interleaved unit must not land between
            # this tile's matmuls (in-order PE would deadlock on the slot)
            psq = psA.tile([P, TCH], F32, tag="aps", name="psq")
            for cc in range(8):
                nc.tensor.matmul(
                    psq[:],
                    wqk_sb[:, cc, j * P:(j + 1) * P],
                    xT[:, cc, ts0:ts0 + TCH],
                    start=(cc == 0), stop=(cc == 7))
            # t1 lives across the yield: with ~12 rotated units in flight the
            # slot pool must be deep enough that reuse never overtakes readers
            t1 = ptmp.tile([P, TCH], F32R, tag="t1", name="t1", bufs=14)
            nc.vector.tensor_scalar_add(t1[:], psq[:], bias_sb[:, j:j + 1])
            yield
            psw = psA.tile([P, TCH], F32, tag="aps", name="psw")
            nc.tensor.matmul(psw[:], perm_sb[:], t1[:], start=True, stop=True)
            qtmp = ptmp.tile([P, TCH], F32, tag="qtmp", name="qtmp", bufs=3)
            nc.vector.tensor_mul(qtmp[:], t1[:], cos_sb[:, ts0:ts0 + TCH])
            swp = ptmp.tile([P, TCH], F32, tag="swp", name="swp", bufs=3)
            nc.vector.tensor_mul(swp[:], psw[:], sin_sb[:, ts0:ts0 + TCH])
            nc.gpsimd.tensor_tensor(
                qkT[:, j, ts0:ts0 + TCH], qtmp[:], swp[:], mybir.AluOpType.add)
            yield

        # ---- phase A unit: one V projection 128-t block
        def gen_v_i(tcn, i):
            ts0 = tcn * TCH
            ti = tcn * (TCH // P) + i
            psv = psA.tile([P, DL], F32, tag="aps", name="psv")
            for cc in range(8):
                nc.tensor.matmul(
                    psv[:],
                    xT[:, cc, ts0 + i * P:ts0 + (i + 1) * P],
                    wv_sb[:, cc, :],
                    start=(cc == 0), stop=(cc == 7))
            nc.vector.tensor_tensor(
                vg[:, ti, :, 0:64],
                psv.rearrange("p (h e) -> p h e", e=64),
                bias_sb[:, 8:8 + DL].rearrange("p (h e) -> p h e", e=64),
                mybir.AluOpType.add)
            yield

        # ---- phase C unit: output projection for one 128-t block
        def gen_c_ti(ti):
            # the finishers writing yT for this chunk must be emitted first
            c = ti // 4
            while not all((c, g) in fin_created for g in range(4)):
                yield  # spin until the finishers exist (safe: only final drain)
            for g in range(4):
                force((c, "fin", g))
            for n in range(2):
                if ti >= 12 and (2 * ti + n) % 2:
                    # tail: the score pipeline is done, borrow its slots so
                    # consecutive psp tiles rotate over 4 banks instead of 2
                    psp = psS.tile([P, TCH], F32, tag="ps", name="psp")
                else:
                    psp = psA.tile([P, TCH], F32, tag="aps", name="psp")
                for g in range(4):
                    nc.tensor.matmul(
                        psp[:],
                        yT[:, g, ti * P:(ti + 1) * P],
                        wp_sb[:, g, n * TCH:(n + 1) * TCH],
                        start=(g == 0), stop=(g == 3))
                ost = post.tile([P, TCH], F32, tag="ost", name="ost")
                nc.scalar.copy(ost[:], psp[:])
                nc.sync.dma_start(
                    out[ti * P:(ti + 1) * P, n * TCH:(n + 1) * TCH], ost[:])
                if n == 0:
                    yield
            yield

        # ---- deferred group finisher: normalize + transpose y -> yT
        def gen_finish(qc, g, y2):
            ysbs = []
            for j in range(4):
                jj = j % 2
                y2t = y2[j // 2]
                y2v = y2t.rearrange("p a (h e) -> p a h e", e=65)
                rcp = prcp.tile([P, 2], F32, tag="rcp", name="rcp")
                nc.vector.reciprocal(
                    rcp[:],
                    y2v[:, jj, :, 64:65].rearrange("p h e -> p (h e)"))
                y_sb = pys.tile([P, P], BF16, tag="ysb", name="ysb", bufs=6)
                for h in range(2):
                    nc.vector.tensor_scalar_mul(
                        y_sb[:, h * 64:(h + 1) * 64],
                        y2t[:, jj, h * 65:h * 65 + 64],
                        rcp[:, h:h + 1])
                ysbs.append(y_sb)
                if j == 1:
                    yield
            for j in range(4):
                psT = psS.tile([P, P], BF16, tag="ps", name="psT")
                nc.tensor.matmul(psT[:], ysbs[j][:], identb[:],
                                 is_transpose=True, skip_group_check=True)
                nc.vector.tensor_copy(
                    yT[:, g, (4 * qc + j) * P:(4 * qc + j + 1) * P], psT[:])
                if j in (1, 3):
                    yield

        # ---- work-unit queue (generators), ratio-paced by the B kb loop
        from collections import deque
        pending = deque()  # items: (kind, gen)
        unit_map = {}      # (tcn, kind, idx) -> item, for targeted forcing
        fin_created = set()
        fill_state = {"slots": 1, "credit": 0.0, "steps": 0}

        executing = set()

        def _step():
            for _ in range(len(pending)):
                item = pending[0]
                if id(item) in executing:
                    pending.rotate(-1)
                    continue
                executing.add(id(item))
                try:
                    next(item[1])
                    ok = True
                except StopIteration:
                    ok = False
                finally:
                    executing.discard(id(item))
                if not ok:
                    pending.popleft()
                    continue
                pending.rotate(-1)  # round-robin across units
                clk["pe"] += STEP_PE_COST.get(item[0], 0.9)
                if item[0] == "c":
                    clk["act"] = max(clk["act"], clk["pe"]) + 0.61
                return True
            return False

        import os
        _margin = float(os.environ.get("K_MARGIN", "0.3"))
        # virtual emission-time clocks (us) for demand-driven fill pacing
        clk = {"pe": 0.0, "act": 0.0}
        STEP_PE_COST = {"qk": 1.0, "v": 1.7, "c": 0.95, "fin": 0.15}

        def fill():
            # emit deferred work until PE's backlog covers ACT's frontier
            while clk["act"] > clk["pe"] + _margin and pending:
                if not _step():
                    break

        def push(kind, nsteps, gen, key=None):
            item = (kind, gen)
            pending.append(item)
            fill_state["steps"] += nsteps
            if key is not None:
                unit_map[key] = item

        def force(key):
            # emit a specific unit to completion now (producers a group
            # ahead); interleave each of its steps with another pending step
            # so its cross-engine chains (psq->t1->psw) get breathing room
            item = unit_map.pop(key, None)
            if item is None or item not in pending:
                return
            pending.remove(item)
            while True:
                try:
                    next(item[1])
                except StopIteration:
                    break
                clk["pe"] += STEP_PE_COST.get(item[0], 0.9)
                _step()

        def drain_finishers():
            fins = [it for it in pending if it[0] == "fin"]
            for it in fins:
                pending.remove(it)
                while True:
                    try:
                        next(it[1])
                    except StopIteration:
                        break
                    _step()

        def drain():
            while pending:
                _step()

        # ---- phase B group: one (q-chunk, head-pair)
        def emit_b_group(qc, g):
            nkb = 4 * qc + 4
            pts = [None] * nkb
            y2 = []

            def emit_s(kb):
                m = kb - 4 * qc
                if m >= 0:
                    force((qc, "v", m))
                off = max(m, 0) * P
                pss = psS.tile([P, 2 * TCH], F32, tag="ps", name="pss")
                for h in range(2):
                    pb = h * 64
                    nc.tensor.matmul(
                        pss[:, h * TCH + off:(h + 1) * TCH],
                        qkT[pb:pb + 64, 4 + g, kb * P:(kb + 1) * P],
                        qkT[pb:pb + 64, g, qc * TCH + off:(qc + 1) * TCH],
                        start=True, stop=True, skip_group_check=True)
                pt = ppt.tile([P, 2 * TCH], BF16, tag="pt", name="pt")
                pts[kb] = pt
                pv = pss.rearrange("p (h w) -> p h w", w=TCH)
                ptv = pt.rearrange("p (h w) -> p h w", w=TCH)
                cols = 2 * (TCH - off)
                clk["pe"] += cols * 0.0004167
                clk["act"] = max(clk["act"], clk["pe"]) + \
                    cols * 0.0008333 + 0.185
                nc.scalar.activation(ptv[:, :, off:], pv[:, :, off:], Exp)
                if m >= 0:
                    nc.vector.tensor_mul(
                        ptv[:, :, m * P:(m + 1) * P],
                        ptv[:, :, m * P:(m + 1) * P],
                        dmask_sb.to_broadcast((P, 2, P)))
                if dbg and qc == 0 and g == 0 and kb == 0:
                    nc.scalar.dma_start(dbg["pt00"], pt[:])

            def alloc_y2():
                # Whole-tile start=True zeroing matmul per y2 bank: the
                # executor's start_tensor_calc marks the full 2KB zero-region
                # pending-zero, so per-region starts would clobber siblings.
                for nm in ("y2a", "y2b"):
                    y2t = psY.tile([P, 2, 130], F32, tag="y2", name=nm)
                    nc.tensor.matmul(
                        y2t.rearrange("p a b -> p (a b)"), z128[:], z260[:],
                        start=True, stop=False, skip_group_check=True)
                    y2.append(y2t)

            def emit_pv(kb):
                m = kb - 4 * qc
                clk["pe"] += (4 - max(m, 0)) * 2 * 65 * 0.0004167
                for j in range(max(m, 0), 4):
                    qj = 4 * qc + j
                    for h in range(2):
                        nc.tensor.matmul(
                            y2[j // 2][:, j % 2, h * 65:(h + 1) * 65],
                            pts[kb][:, h * TCH + j * P:h * TCH + (j + 1) * P],
                            vg[:, kb, 2 * g + h],
                            start=False, stop=(kb == qj),
                            skip_group_check=True)

            emit_s(0)
            emit_s(1)
            fill()
            # finisher of the previous group must be fully emitted before its
            # y2 slots are reallocated below (emission-order slot reuse)
            drain_finishers()
            alloc_y2()
            for kb in range(2, nkb):
                emit_s(kb)
                emit_pv(kb - 2)
                fill()
            emit_pv(nkb - 2)
            emit_pv(nkb - 1)
            if dbg and qc == 0 and g == 0:
                for half in range(2):
                    ytmp = ptmp.tile([P, 2, 130], F32, tag="ydmp", name="ytmp")
                    nc.vector.tensor_copy(ytmp[:], y2[half][:])
                    nc.scalar.dma_start(dbg[f"y2{half}"], ytmp[:])
            item = ("fin", gen_finish(qc, g, tuple(y2)))
            pending.appendleft(item)
            unit_map[(qc, "fin", g)] = item
            fin_created.add((qc, g))
            fill_state["steps"] += 4

        # ---- schedule: A(0); B(qc) x4 with A(qc+1)/C units interleaved
        for j in range(8):
            push("qk", 2, gen_qk_j(0, j), key=(0, "qk", j))
        for i in range(4):
            push("v", 1, gen_v_i(0, i), key=(0, "v", i))

        def force_pair(qc_, g_):
            force((qc_, "qk", g_))
            force((qc_, "qk", 4 + g_))

        # group order: qc-major through B(1); B(2)/B(3) optionally interleaved
        # so B(3)'s ACT-bound stretches share the C/finisher fill supply
        if os.environ.get("K_ORDER", "major") == "il23":
            tail_order = [(2, 0), (3, 0), (2, 1), (3, 1),
                          (2, 2), (3, 2), (2, 3), (3, 3)]
        else:
            tail_order = [(2, g) for g in range(4)] + [(3, g) for g in range(4)]
        order = [(0, g) for g in range(4)] + [(1, g) for g in range(4)] + \
            tail_order
        cdist = os.environ.get("K_CDIST", "a")
        cpush = {"a": {(2, 0): range(0, 4), (3, 0): range(4, 12)},
                 "b": {(2, 0): range(0, 8), (3, 0): range(8, 12)},
                 "c": {(2, 0): range(0, 8), (2, 3): range(8, 12)},
                 "d": {(2, 0): range(0, 4), (2, 2): range(4, 8),
                       (3, 1): range(8, 12)}}[cdist]
        fill_state["slots"] = sum(4 * q + 3 for q, _ in order)
        fill_state["credit"] = 0.0
        for idx, (qc, g) in enumerate(order):
            if (qc, g) in ((0, 0), (1, 0), (2, 0)):
                tcn = qc + 1
                for j in range(8):
                    push("qk", 2, gen_qk_j(tcn, j), key=(tcn, "qk", j))
                for i in range(4):
                    push("v", 1, gen_v_i(tcn, i), key=(tcn, "v", i))
            for ti in cpush.get((qc, g), ()):
                push("c", 2, gen_c_ti(ti))
            force_pair(qc, g)  # usually a no-op (forced a group ahead)
            if idx + 1 < len(order):
                force_pair(*order[idx + 1])
            emit_b_group(qc, g)
        for ti in range(12, 16):
            push("c", 2, gen_c_ti(ti))
        drain()
        if dbg:
            nc.scalar.dma_start(dbg["xT"], xT[:])
            nc.scalar.dma_start(dbg["qkT"], qkT[:])
            nc.scalar.dma_start(dbg["vsb"], vsb[:])
            nc.scalar.dma_start(dbg["yT"], yT[:])


def build_nc():
    nc = bacc.Bacc("TRN2", target_bir_lowering=False, debug=False)
    xb = nc.dram_tensor("xb", [T, C], BF16, kind="ExternalInput").ap()
    wqk = nc.dram_tensor("wqk", [C, 2 * DL], BF16, kind="ExternalInput").ap()
    wv = nc.dram_tensor("wv", [C, DL], BF16, kind="ExternalInput").ap()
    wp = nc.dram_tensor("wp", [DL, C], BF16, kind="ExternalInput").ap()
    cos2 = nc.dram_tensor("cos2", [P, T], F32, kind="ExternalInput").ap()
    sin2 = nc.dram_tensor("sin2", [P, T], F32, kind="ExternalInput").ap()
    bias = nc.dram_tensor("bias", [P, 8 + DL], F32, kind="ExternalInput").ap()
    dmask = nc.dram_tensor("dmask", [P, P], BF16, kind="ExternalInput").ap()
    perm = nc.dram_tensor("perm", [P, P], F32, kind="ExternalInput").ap()
    out = nc.dram_tensor("out", [T, C], F32, kind="ExternalOutput").ap()
    with tile.TileContext(nc) as tc:
        _emit(tc, xb, wqk, wv, wp, cos2, sin2, bias, dmask, perm, out)
    nc.compile()
    return nc


def rope_tables():
    inv_freq = 1.0 / (ROPE_BASE ** (np.arange(0, D, 2, dtype=np.float64) / D))
    t = np.arange(T, dtype=np.float64)
    freqs = np.outer(t, inv_freq)                      # [T, 32]
    emb = np.concatenate([freqs, freqs], axis=-1)      # [T, 64]
    cosT = np.cos(emb).T.astype(np.float32)            # [64, T]
    sinT = np.sin(emb).T.astype(np.float32)
    cos2 = np.tile(cosT, (2, 1)).copy()                # [128, T]
    sin2 = np.tile(sinT, (2, 1)).copy()
    return cos2, sin2


def perm_matrix():
    pm = np.zeros((P, P), dtype=np.float32)
    for base in (0, 64):
        for d in range(32):
            pm[base + d + 32, base + d] = -1.0       # rot_half: -x2 into top
            pm[base + d, base + d + 32] = 1.0        # +x1 into bottom
    return pm


def diag_mask():
    import ml_dtypes
    k = np.arange(P)[:, None]
    q = np.arange(P)[None, :]
    return np.ascontiguousarray((k <= q).astype(ml_dtypes.bfloat16))


def host_inputs(x, W_qkv, b_qkv, W_proj, b_proj):
    import ml_dtypes
    x = np.asarray(x, dtype=np.float32)
    W_qkv = np.asarray(W_qkv, dtype=np.float32)
    b_qkv = np.asarray(b_qkv, dtype=np.float32)
    W_proj = np.asarray(W_proj, dtype=np.float32)
    scale = 1.0 / math.sqrt(D)
    cos2, sin2 = rope_tables()
    dm = diag_mask()
    pm = perm_matrix()
    in_maps = []
    for core in range(NCORES):
        b = core // 2
        hg = core % 2
        s = hg * DL
        wq = W_qkv[:, s:s + DL] * scale
        wk = W_qkv[:, C + s:C + s + DL]
        wqk = np.ascontiguousarray(
            np.concatenate([wq, wk], axis=1)).astype(ml_dtypes.bfloat16)
        wv = np.ascontiguousarray(
            W_qkv[:, 2 * C + s:2 * C + s + DL]).astype(ml_dtypes.bfloat16)
        wp = np.ascontiguousarray(
            W_proj[s:s + DL, :]).astype(ml_dtypes.bfloat16)
        bq = b_qkv[s:s + DL] * scale
        bk = b_qkv[C + s:C + s + DL]
        bv = b_qkv[2 * C + s:2 * C + s + DL]
        bqk = np.concatenate([bq, bk]).reshape(8, P).T          # [128, 8]
        bvb = np.tile(bv[None, :], (P, 1))                      # [128, 512]
        bias = np.ascontiguousarray(
            np.concatenate([bqk, bvb], axis=1).astype(np.float32))
        in_maps.append({
            "xb": np.ascontiguousarray(x[b]).astype(ml_dtypes.bfloat16),
            "wqk": wqk, "wv": wv, "wp": wp,
            "cos2": cos2, "sin2": sin2, "bias": bias, "dmask": dm,
            "perm": pm,
        })
    return in_maps


_NC_CACHE = {}


def run(in_maps, **kwargs):
    if "nc" not in _NC_CACHE:
        _NC_CACHE["nc"] = build_nc()
    return run_bass_kernel_spmd(
        _NC_CACHE["nc"], in_maps, core_ids=list(range(NCORES)), **kwargs)


def kernel(x, W_qkv, b_qkv, W_proj, b_proj, **extra):
    in_maps = host_inputs(x, W_qkv, b_qkv, W_proj, b_proj)
    res = run(in_maps)
    b_proj = np.asarray(b_proj, dtype=np.float32)
    out = np.empty((B, T, C), dtype=np.float32)
    for b in range(B):
        out[b] = res.results[2 * b]["out"] + res.results[2 * b + 1]["out"] + b_proj
    return out


# revision 77
# speedup vs baseline: 1.0742x; 1.0742x over previous
"""Trainium2 Bass kernel for multi-head causal attention with RoPE.

Problem: x[4,2048,1024] -> MHA(16 heads, head_dim 64, RoPE, causal) -> [4,2048,1024]

Sharding: 8 cores = 4 batches x 2 head-groups (8 heads each, Megatron-style).
Each core computes a partial [T, C] projection output for its batch; the host
sums the two head-group partials per batch and adds b_proj.

Per-core dataflow (all matmul moving operands are bf16 -> 1 cycle/row at any
width; stationary reloads are free in the cost model):
  - x is fed as bf16 and transposed HBM->SBUF by the DMA xbar (no PE/DVE cost)
  - Q^T/K^T in [d, t] layout (head-pair tiles on 128 partitions), RoPE via a
    host-precomputed perm matrix on PE + cos/sin muls on DVE, add on Pool;
    1/sqrt(64) folded into W_q host-side; result stored bf16
  - V in natural [t, d] layout bf16, head-major with a 65th column == 1.0
  - scores S^T = K Q^T per (head pair, 512-q chunk, 128-k block), causal
    skipping at 128-col granularity (ragged diagonal), batched exp on ACT,
    diagonal blocks masked post-exp on DVE (bf16 4x mode)
  - P@V in y[q, d] orientation: stationary = P^T block (free reload), moving =
    V65 (ap=65, denominator rides along in column 64); normalize with
    reciprocal + per-partition scalar mul; transpose y back with bf16
    identity (1 cyc/row); Pool copies y^T to SBUF
  - output projection y^T @ W_proj per 128-t chunk in two 512-col halves
  - phases A (proj), B (attention), C (out proj) are interleaved at work-unit
    granularity so B's ACT-bound exp stretches are covered by A/C PE work
"""

import math
import sys

import numpy as np

if "/opt/trn_rl_repo" not in sys.path:
    sys.path.insert(0, "/opt/trn_rl_repo")

import concourse.bass as bass
import concourse.tile as tile
from concourse import bacc
from concourse import mybir
from concourse.bass_utils import run_bass_kernel_spmd
from concourse.masks import make_identity

B, T, C = 4, 2048, 1024
NH, D = 16, 64
HL = 8              # local heads per core
DL = HL * D         # 512
NCORES = 8
P = 128
TCH = 512           # t-chunk width in phase A / q-chunk width in phase B
NTC = T // TCH
ROPE_BASE = 10000.0

F32 = mybir.dt.float32
F32R = mybir.dt.float32r
BF16 = mybir.dt.bfloat16
Exp = mybir.ActivationFunctionType.Exp


def _emit(tc, xb, wqk, wv, wp, cos2, sin2, bias, dmask, perm, out, dbg=None):
    nc = tc.nc
    with tc.tile_pool(name="pers", bufs=1) as pers, \
         tc.tile_pool(name="tmp", bufs=2) as ptmp, \
         tc.tile_pool(name="ptp", bufs=6) as ppt, \
         tc.tile_pool(name="ysp", bufs=4) as pys, \
         tc.tile_pool(name="rcp", bufs=2) as prcp, \
         tc.tile_pool(name="ost", bufs=2) as post, \
         tc.tile_pool(name="psa", bufs=2, space="PSUM") as psA, \
         tc.tile_pool(name="pss", bufs=2, space="PSUM") as psS, \
         tc.tile_pool(name="psy", bufs=2, space="PSUM") as psY:
        xT = pers.tile([P, 8, T], BF16)        # x^T: [c%128... c = cc*128+p]
        qkT = pers.tile([P, 8, T], BF16)       # j 0-3: Q pairs, 4-7: K pairs
        vsb = pers.tile([P, 16, HL * 65], BF16)  # [t%128, t//128, h*65 + e]
        yT = pers.tile([P, 4, T], BF16)        # [d2, g, t]
        wqk_sb = pers.tile([P, 8, 2 * DL], BF16)
        wv_sb = pers.tile([P, 8, DL], BF16)
        wp_sb = pers.tile([P, 4, C], BF16)
        cos_sb = pers.tile([P, T], F32)
        sin_sb = pers.tile([P, T], F32)
        bias_sb = pers.tile([P, 8 + DL], F32)
        dmask_sb = pers.tile([P, 1, P], BF16)  # [k, 1, q]: 1.0 iff k <= q
        perm_sb = pers.tile([P, P], F32R)
        identb = pers.tile([P, P], BF16)
        z128 = pers.tile([1, P], BF16)
        z260 = pers.tile([1, 260], BF16)

        make_identity(nc, identb)
        nc.vector.memset(z128[:], 0.0)
        nc.vector.memset(z260[:], 0.0)
        # DMA priority order: first x chunk + first weights before the rest so
        # PE can start within ~8us (the sim serializes all DMA on one resource)
        nc.sync.dma_start_transpose(xT[:, :, 0:TCH], xb[0:TCH, :])
        nc.sync.dma_start(
            wqk_sb[:, :, 0:DL], wqk[:, 0:DL].rearrange("(o p) n -> p o n", p=P))
        nc.sync.dma_start(cos_sb[:], cos2)
        nc.sync.dma_start(sin_sb[:], sin2)
        nc.sync.dma_start(
            wqk_sb[:, :, DL:], wqk[:, DL:].rearrange("(o p) n -> p o n", p=P))
        nc.sync.dma_start(wv_sb[:], wv.rearrange("(o p) n -> p o n", p=P))
        nc.sync.dma_start(bias_sb[:], bias)
        nc.sync.dma_start_transpose(xT[:, :, TCH:2 * TCH], xb[TCH:2 * TCH, :])
        nc.sync.dma_start(dmask_sb[:], dmask.rearrange("p (o w) -> p o w", o=1))
        nc.gpsimd.dma_start(perm_sb[:], perm)
        nc.sync.dma_start_transpose(
            xT[:, :, 2 * TCH:3 * TCH], xb[2 * TCH:3 * TCH, :])
        nc.sync.dma_start_transpose(
            xT[:, :, 3 * TCH:4 * TCH], xb[3 * TCH:4 * TCH, :])
        nc.sync.dma_start(wp_sb[:], wp.rearrange("(o p) n -> p o n", p=P))
        vg = vsb.rearrange("p a (h e) -> p a h e", e=65)
        nc.vector.memset(vg[:, :, :, 64:65], 1.0)

        # ---- phase A unit: one QK projection column-pair j of one t-chunk
        def gen_qk_j(tcn, j):
            ts0 = tcn * TCH
            # the whole psq accumulation stays in one step: a same-slot
            # allocation from another interleaved unit must not land between
            # this tile's matmuls (in-order PE would deadlock on the slot)
            psq = psA.tile([P, TCH], F32, tag="aps", name="psq")
            for cc in range(8):
                nc.tensor.matmul(
                    psq[:],
                    wqk_sb[:, cc, j * P:(j + 1) * P],
                    xT[:, cc, ts0:ts0 + TCH],
                    start=(cc == 0), stop=(cc == 7))
            # t1 lives across the yield: with ~12 rotated units in flight the
            # slot pool must be deep enough that reuse never overtakes readers
            t1 = ptmp.tile([P, TCH], F32R, tag="t1", name="t1", bufs=14)
            nc.vector.tensor_scalar_add(t1[:], psq[:], bias_sb[:, j:j + 1])
            yield
            psw = psA.tile([P, TCH], F32, tag="aps", name="psw")
            nc.tensor.matmul(psw[:], perm_sb[:], t1[:], start=True, stop=True)
            qtmp = ptmp.tile([P, TCH], F32, tag="qtmp", name="qtmp", bufs=3)
            nc.vector.tensor_mul(qtmp[:], t1[:], cos_sb[:, ts0:ts0 + TCH])
            swp = ptmp.tile([P, TCH], F32, tag="swp", name="swp", bufs=3)
            nc.vector.tensor_mul(swp[:], psw[:], sin_sb[:, ts0:ts0 + TCH])
            nc.gpsimd.tensor_tensor(
                qkT[:, j, ts0:ts0 + TCH], qtmp[:], swp[:], mybir.AluOpType.add)
            yield

        # ---- phase A unit: one V projection 128-t block
        def gen_v_i(tcn, i):
            ts0 = tcn * TCH
            ti = tcn * (TCH // P) + i
            psv = psA.tile([P, DL], F32, tag="aps", name="psv")
            for cc in range(8):
                nc.tensor.matmul(
                    psv[:],
                    xT[:, cc, ts0 + i * P:ts0 + (i + 1) * P],
                    wv_sb[:, cc, :],
                    start=(cc == 0), stop=(cc == 7))
            nc.vector.tensor_tensor(
                vg[:, ti, :, 0:64],
                psv.rearrange("p (h e) -> p h e", e=64),
                bias_sb[:, 8:8 + DL].rearrange("p (h e) -> p h e", e=64),
                mybir.AluOpType.add)
            yield

        # ---- phase C unit: output projection for one 128-t block
        def gen_c_ti(ti):
            # the finishers writing yT for this chunk must be emitted first
            c = ti // 4
            while not all((c, g) in fin_created for g in range(4)):
                yield  # spin until the finishers exist (safe: only final drain)
            for g in range(4):
                force((c, "fin", g))
            for n in range(2):
                if ti >= 12 and (2 * ti + n) % 2:
                    # tail: the score pipeline is done, borrow its slots so
                    # consecutive psp tiles rotate over 4 banks instead of 2
                    psp = psS.tile([P, TCH], F32, tag="ps", name="psp")
                else:
                    psp = psA.tile([P, TCH], F32, tag="aps", name="psp")
                for g in range(4):
                    nc.tensor.matmul(
                        psp[:],
                        yT[:, g, ti * P:(ti + 1) * P],
                        wp_sb[:, g, n * TCH:(n + 1) * TCH],
                        start=(g == 0), stop=(g == 3))
                ost = post.tile([P, TCH], F32, tag="ost", name="ost")
                nc.scalar.copy(ost[:], psp[:])
                nc.sync.dma_start(
                    out[ti * P:(ti + 1) * P, n * TCH:(n + 1) * TCH], ost[:])
                if n == 0:
                    yield
            yield

        # ---- deferred group finisher: normalize + transpose y -> yT
        def gen_finish(qc, g, y2):
            ysbs = []
            for j in range(4):
                jj = j % 2
                y2t = y2[j // 2]
                y2v = y2t.rearrange("p a (h e) -> p a h e", e=65)
                rcp = prcp.tile([P, 2], F32, tag="rcp", name="rcp")
                nc.vector.reciprocal(
                    rcp[:],
                    y2v[:, jj, :, 64:65].rearrange("p h e -> p (h e)"))
                y_sb = pys.tile([P, P], BF16, tag="ysb", name="ysb", bufs=6)
                for h in range(2):
                    nc.vector.tensor_scalar_mul(
                        y_sb[:, h * 64:(h + 1) * 64],
                        y2t[:, jj, h * 65:h * 65 + 64],
                        rcp[:, h:h + 1])
                ysbs.append(y_sb)
                if j == 1:
                    yield
            for j in range(4):
                psT = psS.tile([P, P], BF16, tag="ps", name="psT")
                nc.tensor.matmul(psT[:], ysbs[j][:], identb[:],
                                 is_transpose=True, skip_group_check=True)
                nc.vector.tensor_copy(
                    yT[:, g, (4 * qc + j) * P:(4 * qc + j + 1) * P], psT[:])
                if j in (1, 3):
                    yield

        # ---- work-unit queue (generators), ratio-paced by the B kb loop
        from collections import deque
        pending = deque()  # items: (kind, gen)
        unit_map = {}      # (tcn, kind, idx) -> item, for targeted forcing
        fin_created = set()
        fill_state = {"slots": 1, "credit": 0.0, "steps": 0}

        executing = set()

        def _step():
            for _ in range(len(pending)):
                item = pending[0]
                if id(item) in executing:
                    pending.rotate(-1)
                    continue
                executing.add(id(item))
                try:
                    next(item[1])
                    ok = True
                except StopIteration:
                    ok = False
                finally:
                    executing.discard(id(item))
                if not ok:
                    pending.popleft()
                    continue
                pending.rotate(-1)  # round-robin across units
                clk["pe"] += STEP_PE_COST.get(item[0], 0.9)
                if item[0] == "c":
                    clk["act"] = max(clk["act"], clk["pe"]) + 0.61
                return True
            return False

        import os
        _margin = float(os.environ.get("K_MARGIN", "0.3"))
        # virtual emission-time clocks (us) for demand-driven fill pacing
        clk = {"pe": 0.0, "act": 0.0}
        STEP_PE_COST = {"qk": 1.0, "v": 1.7, "c": 0.95, "fin": 0.15}

        _floor = float(os.environ.get("K_FLOOR", "0.34"))

        def fill():
            slots = max(fill_state["slots"], 1)
            est = 2.0 * len(pending)
            if pending:
                fill_state["credit"] += min(max(est / slots, _floor), 3.0)
            n = int(fill_state["credit"])
            fill_state["credit"] -= n
            for _ in range(n):
                if not _step():
                    break
            fill_state["slots"] -= 1

        def push(kind, nsteps, gen, key=None):
            item = (kind, gen)
            pending.append(item)
            fill_state["steps"] += nsteps
            if key is not None:
                unit_map[key] = item

        def force(key):
            # emit a specific unit to completion now (producers a group
            # ahead); interleave each of its steps with another pending step
            # so its cross-engine chains (psq->t1->psw) get breathing room
            item = unit_map.pop(key, None)
            if item is None or item not in pending:
                return
            pending.remove(item)
            while True:
                try:
                    next(item[1])
                except StopIteration:
                    break
                clk["pe"] += STEP_PE_COST.get(item[0], 0.9)
                _step()

        def drain_finishers():
            fins = [it for it in pending if it[0] == "fin"]
            for it in fins:
                pending.remove(it)
                while True:
                    try:
                        next(it[1])
                    except StopIteration:
                        break
                    _step()

        def drain():
            while pending:
                _step()

        # ---- phase B group: one (q-chunk, head-pair)
        def emit_b_group(qc, g):
            nkb = 4 * qc + 4
            pts = [None] * nkb
            y2 = []

            def emit_s(kb):
                m = kb - 4 * qc
                if m >= 0:
                    force((qc, "v", m))
                off = max(m, 0) * P
                pss = psS.tile([P, 2 * TCH], F32, tag="ps", name="pss")
                for h in range(2):
                    pb = h * 64
                    nc.tensor.matmul(
                        pss[:, h * TCH + off:(h + 1) * TCH],
                        qkT[pb:pb + 64, 4 + g, kb * P:(kb + 1) * P],
                        qkT[pb:pb + 64, g, qc * TCH + off:(qc + 1) * TCH],
                        start=True, stop=True, skip_group_check=True)
                pt = ppt.tile([P, 2 * TCH], BF16, tag="pt", name="pt")
                pts[kb] = pt
                pv = pss.rearrange("p (h w) -> p h w", w=TCH)
                ptv = pt.rearrange("p (h w) -> p h w", w=TCH)
                cols = 2 * (TCH - off)
                clk["pe"] += cols * 0.0004167
                clk["act"] = max(clk["act"], clk["pe"]) + \
                    cols * 0.0008333 + 0.185
                nc.scalar.activation(ptv[:, :, off:], pv[:, :, off:], Exp)
                if m >= 0:
                    nc.vector.tensor_mul(
                        ptv[:, :, m * P:(m + 1) * P],
                        ptv[:, :, m * P:(m + 1) * P],
                        dmask_sb.to_broadcast((P, 2, P)))
                if dbg and qc == 0 and g == 0 and kb == 0:
                    nc.scalar.dma_start(dbg["pt00"], pt[:])

            def alloc_y2():
                # Whole-tile start=True zeroing matmul per y2 bank: the
                # executor's start_tensor_calc marks the full 2KB zero-region
                # pending-zero, so per-region starts would clobber siblings.
                for nm in ("y2a", "y2b"):
                    y2t = psY.tile([P, 2, 130], F32, tag="y2", name=nm)
                    nc.tensor.matmul(
                        y2t.rearrange("p a b -> p (a b)"), z128[:], z260[:],
                        start=True, stop=False, skip_group_check=True)
                    y2.append(y2t)

            def emit_pv(kb):
                m = kb - 4 * qc
                clk["pe"] += (4 - max(m, 0)) * 2 * 65 * 0.0004167
                for j in range(max(m, 0), 4):
                    qj = 4 * qc + j
                    for h in range(2):
                        nc.tensor.matmul(
                            y2[j // 2][:, j % 2, h * 65:(h + 1) * 65],
                            pts[kb][:, h * TCH + j * P:h * TCH + (j + 1) * P],
                            vg[:, kb, 2 * g + h],
                            start=False, stop=(kb == qj),
                            skip_group_check=True)

            emit_s(0)
            emit_s(1)
            fill()
            # finisher of the previous group must be fully emitted before its
            # y2 slots are reallocated below (emission-order slot reuse)
            drain_finishers()
            alloc_y2()
            for kb in range(2, nkb):
                emit_s(kb)
                emit_pv(kb - 2)
                fill()
            emit_pv(nkb - 2)
            emit_pv(nkb - 1)
            if dbg and qc == 0 and g == 0:
                for half in range(2):
                    ytmp = ptmp.tile([P, 2, 130], F32, tag="ydmp", name="ytmp")
                    nc.vector.tensor_copy(ytmp[:], y2[half][:])
                    nc.scalar.dma_start(dbg[f"y2{half}"], ytmp[:])
            item = ("fin", gen_finish(qc, g, tuple(y2)))
            pending.appendleft(item)
            unit_map[(qc, "fin", g)] = item
            fin_created.add((qc, g))
            fill_state["steps"] += 4

        # ---- schedule: A(0); B(qc) x4 with A(qc+1)/C units interleaved
        for j in range(8):
            push("qk", 2, gen_qk_j(0, j), key=(0, "qk", j))
        for i in range(4):
            push("v", 1, gen_v_i(0, i), key=(0, "v", i))

        def force_pair(qc_, g_):
            force((qc_, "qk", g_))
            force((qc_, "qk", 4 + g_))

        # group order: qc-major through B(1); B(2)/B(3) optionally interleaved
        # so B(3)'s ACT-bound stretches share the C/finisher fill supply
        if os.environ.get("K_ORDER", "major") == "il23":
            tail_order = [(2, 0), (3, 0), (2, 1), (3, 1),
                          (2, 2), (3, 2), (2, 3), (3, 3)]
        else:
            tail_order = [(2, g) for g in range(4)] + [(3, g) for g in range(4)]
        order = [(0, g) for g in range(4)] + [(1, g) for g in range(4)] + \
            tail_order
        cdist = os.environ.get("K_CDIST", "a")
        cpush = {"a": {(2, 0): range(0, 4), (3, 0): range(4, 12)},
                 "b": {(2, 0): range(0, 8), (3, 0): range(8, 12)},
                 "c": {(2, 0): range(0, 8), (2, 3): range(8, 12)},
                 "d": {(2, 0): range(0, 4), (2, 2): range(4, 8),
                       (3, 1): range(8, 12)}}[cdist]
        # reserve: keep these C units out of rotation until late B(3), where
        # the exp pipeline otherwise starves the PE
        reserve_tis = set(range(8, 12)) if os.environ.get(
            "K_RESERVE", "1") == "1" else set()
        release_at = int(os.environ.get("K_RELEASE", "14"))
        fill_state["slots"] = sum(4 * q + 3 for q, _ in order)
        fill_state["credit"] = 0.0
        late = []
        for idx, (qc, g) in enumerate(order):
            if (qc, g) in ((0, 0), (1, 0), (2, 0)):
                tcn = qc + 1
                for j in range(8):
                    push("qk", 2, gen_qk_j(tcn, j), key=(tcn, "qk", j))
                for i in range(4):
                    push("v", 1, gen_v_i(tcn, i), key=(tcn, "v", i))
            for ti in cpush.get((qc, g), ()):
                if ti in reserve_tis:
                    late.append(ti)
                else:
                    push("c", 2, gen_c_ti(ti))
            if idx == release_at and late:
                for ti in late:
                    push("c", 2, gen_c_ti(ti))
                late = []
            force_pair(qc, g)  # usually a no-op (forced a group ahead)
            if idx + 1 < len(order):
                force_pair(*order[idx + 1])
            emit_b_group(qc, g)
        for ti in late:
            push("c", 2, gen_c_ti(ti))
        for ti in range(12, 16):
            push("c", 2, gen_c_ti(ti))
        drain()
        if dbg:
            nc.scalar.dma_start(dbg["xT"], xT[:])
            nc.scalar.dma_start(dbg["qkT"], qkT[:])
            nc.scalar.dma_start(dbg["vsb"], vsb[:])
            nc.scalar.dma_start(dbg["yT"], yT[:])


def build_nc():
    nc = bacc.Bacc("TRN2", target_bir_lowering=False, debug=False)
    xb = nc.dram_tensor("xb", [T, C], BF16, kind="ExternalInput").ap()
    wqk = nc.dram_tensor("wqk", [C, 2 * DL], BF16, kind="ExternalInput").ap()
    wv = nc.dram_tensor("wv", [C, DL], BF16, kind="ExternalInput").ap()
    wp = nc.dram_tensor("wp", [DL, C], BF16, kind="ExternalInput").ap()
    cos2 = nc.dram_tensor("cos2", [P, T], F32, kind="ExternalInput").ap()
    sin2 = nc.dram_tensor("sin2", [P, T], F32, kind="ExternalInput").ap()
    bias = nc.dram_tensor("bias", [P, 8 + DL], F32, kind="ExternalInput").ap()
    dmask = nc.dram_tensor("dmask", [P, P], BF16, kind="ExternalInput").ap()
    perm = nc.dram_tensor("perm", [P, P], F32, kind="ExternalInput").ap()
    out = nc.dram_tensor("out", [T, C], F32, kind="ExternalOutput").ap()
    with tile.TileContext(nc) as tc:
        _emit(tc, xb, wqk, wv, wp, cos2, sin2, bias, dmask, perm, out)
    nc.compile()
    return nc


def rope_tables():
    inv_freq = 1.0 / (ROPE_BASE ** (np.arange(0, D, 2, dtype=np.float64) / D))
    t = np.arange(T, dtype=np.float64)
    freqs = np.outer(t, inv_freq)                      # [T, 32]
    emb = np.concatenate([freqs, freqs], axis=-1)      # [T, 64]
    cosT = np.cos(emb).T.astype(np.float32)            # [64, T]
    sinT = np.sin(emb).T.astype(np.float32)
    cos2 = np.tile(cosT, (2, 1)).copy()                # [128, T]
    sin2 = np.tile(sinT, (2, 1)).copy()
    return cos2, sin2


def perm_matrix():
    pm = np.zeros((P, P), dtype=np.float32)
    for base in (0, 64):
        for d in range(32):
            pm[base + d + 32, base + d] = -1.0       # rot_half: -x2 into top
            pm[base + d, base + d + 32] = 1.0        # +x1 into bottom
    return pm


def diag_mask():
    import ml_dtypes
    k = np.arange(P)[:, None]
    q = np.arange(P)[None, :]
    return np.ascontiguousarray((k <= q).astype(ml_dtypes.bfloat16))


def host_inputs(x, W_qkv, b_qkv, W_proj, b_proj):
    import ml_dtypes
    x = np.asarray(x, dtype=np.float32)
    W_qkv = np.asarray(W_qkv, dtype=np.float32)
    b_qkv = np.asarray(b_qkv, dtype=np.float32)
    W_proj = np.asarray(W_proj, dtype=np.float32)
    scale = 1.0 / math.sqrt(D)
    cos2, sin2 = rope_tables()
    dm = diag_mask()
    pm = perm_matrix()
    in_maps = []
    for core in range(NCORES):
        b = core // 2
        hg = core % 2
        s = hg * DL
        wq = W_qkv[:, s:s + DL] * scale
        wk = W_qkv[:, C + s:C + s + DL]
        wqk = np.ascontiguousarray(
            np.concatenate([wq, wk], axis=1)).astype(ml_dtypes.bfloat16)
        wv = np.ascontiguousarray(
            W_qkv[:, 2 * C + s:2 * C + s + DL]).astype(ml_dtypes.bfloat16)
        wp = np.ascontiguousarray(
            W_proj[s:s + DL, :]).astype(ml_dtypes.bfloat16)
        bq = b_qkv[s:s + DL] * scale
        bk = b_qkv[C + s:C + s + DL]
        bv = b_qkv[2 * C + s:2 * C + s + DL]
        bqk = np.concatenate([bq, bk]).reshape(8, P).T          # [128, 8]
        bvb = np.tile(bv[None, :], (P, 1))                      # [128, 512]
        bias = np.ascontiguousarray(
            np.concatenate([bqk, bvb], axis=1).astype(np.float32))
        in_maps.append({
            "xb": np.ascontiguousarray(x[b]).astype(ml_dtypes.bfloat16),
            "wqk": wqk, "wv": wv, "wp": wp,
            "cos2": cos2, "sin2": sin2, "bias": bias, "dmask": dm,
            "perm": pm,
        })
    return in_maps


_NC_CACHE = {}


def run(in_maps, **kwargs):
    if "nc" not in _NC_CACHE:
        _NC_CACHE["nc"] = build_nc()
    return run_bass_kernel_spmd(
        _NC_CACHE["nc"], in_maps, core_ids=list(range(NCORES)), **kwargs)


def kernel(x, W_qkv, b_qkv, W_proj, b_proj, **extra):
    in_maps = host_inputs(x, W_qkv, b_qkv, W_proj, b_proj)
    res = run(in_maps)
    b_proj = np.asarray(b_proj, dtype=np.float32)
    out = np.empty((B, T, C), dtype=np.float32)
    for b in range(B):
        out[b] = res.results[2 * b]["out"] + res.results[2 * b + 1]["out"] + b_proj
    return out
